# revision 35
# baseline (speedup 1.0000x reference)
"""HDC sigmoid-attention kernel for Trainium2 (8 NeuronCores).

Problem: out = causal_sigmoid_attn(q, k, v) where q/k/v = x * sign_vec(bv_*),
x: [4, 4096, 1024] f32.  Returns (out, k, v) like the reference.

Sharding: 8 cores = 4 batches x 2 row-parity groups.  Core (b, h) handles
batch b, rows {t : t % 2 == h}.  Row-parity interleaving makes the causal
work profile identical on every core, so one SPMD program serves all 8.

Per core: 2048 rows as 8 t-blocks (J=0..7) of 256 local rows; t-block J
covers global rows {512J + 2m + h}.  Causal extent of block J is s-chunks
0..4J+3 (chunk = 128 s values); the top 4 chunks (mi = c-4J in 0..3) are
diagonal and get a 0/1 mask (host-precomputed, J-independent thanks to the
parity trick).  For mi >= 2 the lower t-half (local rows 0..127) is
entirely above the diagonal -> masked zero, so those chunks compute only
the t 128:256 half in mm1/ACT and skip the tt=0 matmuls in mm2.  The tt=0
output accumulators therefore stop at chunk 4J+1 and drain 2 chunks early,
hiding the PSUM->SBUF copy behind the diagonal chunks' compute.

Matmul 1 (scores^T), fp8 e4m3 DoubleRow (K=256/instr, 2x PE rate):
  psum[s=128, t=256] += sum_j kT[d pair j, s].T @ qT[d pair j, t]
Sigmoid(0.125 * scores) on ACT (psum -> bf16 sbuf), mask-mul on DVE for
diagonal chunks.
Matmul 2 (out), bf16: psum[t=128, d=512] += attnT[s,t].T @ v[s,d]

kT (4MB fp8) and v (8MB bf16) fully SBUF-resident; qT fp8 streamed per
t-block.  Input DMAs are spread across 4 engine queues (qT/out on SP,
kT on ACT, v on DVE, masks on GpSimd) so nothing serializes behind the
first block's inputs.  fp8 scores + bf16 second matmul give rel err
~1.6e-2 vs the f32 reference (gate 2e-2).
"""

import numpy as np
import ml_dtypes

import concourse.bass as bass
import concourse.bacc as bacc
import concourse.mybir as mybir
import concourse.tile as tile
from concourse.bass_utils import run_bass_kernel_spmd

B, T, D = 4, 4096, 1024
P = 128
NJ = 8          # t-blocks per core
TB = 256        # local rows per t-block
NC = 32         # s-chunks per batch

F32 = mybir.dt.float32
BF16 = mybir.dt.bfloat16
FP8 = mybir.dt.float8e4
NP_FP8 = ml_dtypes.float8_e4m3
NP_BF16 = ml_dtypes.bfloat16
DR = mybir.MatmulPerfMode.DoubleRow

_nc_cache = {}
TRACE = False  # set True (e.g. from test.py) to collect an NTFF profile


def _build_nc(reps=1):
    nc = bacc.Bacc("TRN2", debug=False, target_bir_lowering=False, num_devices=8)

    qT_d = nc.dram_tensor("qT", [NJ, P, 8, TB], FP8, kind="ExternalInput")
    # kT/v pre-paired on host (contiguous 256/512KB transfers)
    kT_d = nc.dram_tensor("kT", [NC // 2, P, 2, 8, 128], FP8, kind="ExternalInput")
    v_d = nc.dram_tensor("v", [NC // 2, P, 2048], BF16, kind="ExternalInput")
    mk_d = nc.dram_tensor("masks", [4, P, TB], BF16, kind="ExternalInput")
    out_d = nc.dram_tensor("out_loc", [2048, D], BF16, kind="ExternalOutput")

    with tile.TileContext(nc) as tc:
        with (
            tc.tile_pool(name="vres", bufs=1) as vpool,
            tc.tile_pool(name="kres", bufs=1) as krespool,
            tc.tile_pool(name="qt", bufs=8) as qpool,
            tc.tile_pool(name="attn", bufs=8) as apool,
            tc.tile_pool(name="mask", bufs=1) as mpool,
            tc.tile_pool(name="ostage", bufs=16) as opool,
            tc.tile_pool(name="ps_s", bufs=4, space=bass.MemorySpace.PSUM) as pspool,
            tc.tile_pool(name="ps_o", bufs=1, space=bass.MemorySpace.PSUM) as popool,
        ):
            # All inputs are issued up front, round-robin across the three
            # DMA-capable queues (SP / ACT / GpSimd, each ~100GB/s FIFO),
            # in NEED order: masks+qt0+k0/v0 first, then per-block k/v
            # pairs, so no block ever waits on a queue hauling later data.
            # Inputs round-robin over SP+GpSimd only (~100GB/s FIFO each);
            # the ACT queue carries NO dmas so sigmoids are never delayed
            # by descriptor issue or flush semaphore waits.
            qs2 = [nc.sync, nc.gpsimd]
            qi = [0]

            def issue(out, in_):
                qs2[qi[0] % 2].dma_start(out=out, in_=in_)
                qi[0] += 1

            masks = []
            for mi in range(4):
                mt = mpool.tile([P, TB], BF16, tag=f"mask{mi}")
                masks.append(mt)
            qt_tiles = []
            for J in range(NJ):
                qt = qpool.tile([P, 8, TB], FP8, tag="qt", name=f"qt{J}")
                qt_tiles.append(qt)
            k_pairs = []
            for p in range(NC // 2):
                kp = krespool.tile([P, 2, 8, 128], FP8, tag=f"kp{p}", name=f"kp{p}")
                k_pairs.append(kp)
            v_pairs = []
            for p in range(NC // 2):
                vp = vpool.tile([P, 2048], BF16, tag=f"vp{p}", name=f"vp{p}")
                v_pairs.append(vp)

            # need-ordered issue; the first chunk's j=0 operands go first in
            # tiny DMAs so the PE can start ~3us earlier
            issue(qt_tiles[0][:, 0:2, :], qT_d[0, :, 0:2, :])       # sync, 64KB
            issue(k_pairs[0][:, 0, 0:2, :], kT_d[0, :, 0, 0:2, :])  # gpsimd, 32KB
            issue(qt_tiles[0][:, 2:8, :], qT_d[0, :, 2:8, :])
            issue(k_pairs[0][:, 0, 2:8, :], kT_d[0, :, 0, 2:8, :])
            issue(qt_tiles[1][:, 0:4, :], qT_d[1, :, 0:4, :])
            issue(masks[0][:], mk_d[0])
            issue(v_pairs[0][:, 0:1024], v_d[0, :, 0:1024])
            issue(qt_tiles[1][:, 4:8, :], qT_d[1, :, 4:8, :])
            issue(masks[1][:], mk_d[1])
            issue(k_pairs[0][:, 1, :, :], kT_d[0, :, 1, :, :])
            issue(v_pairs[0][:, 1024:2048], v_d[0, :, 1024:2048])
            issue(masks[2][:], mk_d[2])
            issue(masks[3][:], mk_d[3])
            issue(k_pairs[1][:], kT_d[1])
            issue(v_pairs[1][:], v_d[1])
            for J in range(1, NJ):
                if J >= 2:
                    issue(qt_tiles[J][:], qT_d[J])
                for p in (2 * J, 2 * J + 1):
                    issue(k_pairs[p][:], kT_d[p])
                    issue(v_pairs[p][:], v_d[p])

            def get_qt(J):
                return qt_tiles[J]

            def get_k(c):
                # [P, 2, 128] d-pair slice view resolved at use site
                return k_pairs[c // 2]

            def get_v(c):
                return v_pairs[c // 2]

            import contextlib
            rep_ctx = tc.For_i(0, reps, 1) if reps > 1 else contextlib.nullcontext()
            with rep_ctx:
                _kernel_body(nc, tc, get_qt, get_k, get_v, out_d, masks,
                             qpool, apool, opool, pspool, popool)

    nc.compile()
    return nc


def _kernel_body(nc, tc, get_qt, get_k, get_v, out_d, masks,
                 qpool, apool, opool, pspool, popool):
    for J in range(NJ):
        qt = get_qt(J)
        ns = 4 * J + 4
        ns0 = 4 * J + 2       # chunks feeding the tt=0 (lower t-half) accs
        accs = []
        for i in range(4):
            acc_t = popool.tile([P, 512], F32, tag=f"acc{i}", name=f"acc{i}_{J}")
            accs.append(acc_t)

        def flush(tt):
            ot = opool.tile([P, 1024], BF16, tag="ostage", name=f"ot{tt}_{J}")
            for dd in range(2):
                # dd=0 on DVE, dd=1 on ACT: parallel copies keep the DVE free
                # for the diagonal chunks' mask-muls right behind this point
                if dd == 0:
                    nc.vector.tensor_copy(
                        ot[:, dd * 512:(dd + 1) * 512], accs[tt * 2 + dd][:]
                    )
                else:
                    nc.scalar.activation(
                        ot[:, dd * 512:(dd + 1) * 512], accs[tt * 2 + dd][:],
                        mybir.ActivationFunctionType.Copy,
                    )
                # gpsimd/SP alternately; both engines are idle post-pack, so
                # the engine-side wait on the copy semaphore costs nothing.
                # Per-half DMAs shorten the copy->write tail on the last block.
                eng = nc.gpsimd if (J + tt + dd) % 2 == 0 else nc.sync
                eng.dma_start(
                    out=out_d[J * TB + tt * 128: J * TB + (tt + 1) * 128,
                              dd * 512:(dd + 1) * 512],
                    in_=ot[:, dd * 512:(dd + 1) * 512],
                )

        for ci in range(ns):
            c = ci
            kt = get_k(c)
            cc = c % 2
            mi = c - 4 * J
            half = mi >= 2        # lower t-half fully masked -> skip it
            lo = 128 if half else 0
            ps = pspool.tile([P, TB], F32, tag="scores")
            for j in range(4):
                nc.tensor.matmul(
                    ps[:, lo:TB],
                    kt[:, cc, 2 * j:2 * j + 2, :],
                    qt[:, 2 * j:2 * j + 2, lo:TB],
                    start=(j == 0),
                    stop=(j == 3),
                    perf_mode=DR,
                )
            at = apool.tile([P, TB], BF16, tag="attn")
            nc.scalar.activation(
                at[:, lo:TB], ps[:, lo:TB],
                mybir.ActivationFunctionType.Sigmoid,
                scale=0.125,
            )
            if mi >= 0:
                nc.vector.tensor_mul(
                    at[:, lo:TB], at[:, lo:TB], masks[mi][:, lo:TB]
                )
            voff = cc * 1024
            for tt in range((1 if half else 0), 2):
                for dd in range(2):
                    nc.tensor.matmul(
                        accs[tt * 2 + dd][:],
                        at[:, tt * 128:(tt + 1) * 128],
                        get_v(c)[:, voff + dd * 512:voff + (dd + 1) * 512],
                        start=(ci == 0),
                        stop=(ci == (ns0 - 1 if tt == 0 else ns - 1)),
                    )
            if ci == ns0 - 1:
                flush(0)      # tt=0 accs are complete; drain them now
        flush(1)


def _get_nc(reps=1):
    key = ("nc", reps)
    if key not in _nc_cache:
        _nc_cache[key] = _build_nc(reps)
    return _nc_cache[key]


def _sign_vec(w):
    w = np.asarray(w, np.float32)
    alpha = np.float32(np.mean(np.abs(w), dtype=np.float32))
    hard = (alpha * np.sign(w)).astype(np.float32)
    hard = np.where(hard == 0, alpha, hard).astype(np.float32)
    return hard


def _rows_of(h):
    l = np.arange(2048)
    return 512 * (l // 256) + 2 * (l % 256) + h


def _masks_of(h):
    m = np.arange(TB)[None, :]      # local row in t-block
    p = np.arange(P)[:, None]       # s within chunk
    out = np.empty((4, P, TB), np.float32)
    for mi in range(4):
        out[mi] = ((2 * m + h) >= (128 * mi + p)).astype(np.float32)
    return out


def kernel(x, bv_q, bv_k, bv_v):
    x = np.ascontiguousarray(np.asarray(x, np.float32))
    sq = _sign_vec(bv_q)
    sk = _sign_vec(bv_k)
    sv = _sign_vec(bv_v)

    q_full = (x * sq).astype(np.float32)
    k_full = (x * sk).astype(np.float32)
    v_full = (x * sv).astype(np.float32)

    nc = _get_nc()
    rows = {h: _rows_of(h) for h in range(2)}
    mks = {h: _masks_of(h) for h in range(2)}

    in_maps = []
    for core in range(8):
        b, h = core // 2, core % 2
        qrows = q_full[b][rows[h]]                       # [2048, 1024]
        qT_host = np.ascontiguousarray(
            qrows.reshape(NJ, TB, 8, P).transpose(0, 3, 2, 1)
        )  # [NJ, P(di), 8(do), TB]
        kT_host = np.ascontiguousarray(
            k_full[b].reshape(NC // 2, 2, P, 8, P).transpose(0, 4, 1, 3, 2)
        )  # [pair, P(di), 2(cc), 8(do), 128(s)]
        v_host = np.ascontiguousarray(
            v_full[b].reshape(NC // 2, 2, P, 1024).transpose(0, 2, 1, 3)
            .reshape(NC // 2, P, 2048)
        )  # [pair, P(s in chunk), (cc,d)]
        in_maps.append({
            "qT": qT_host.astype(NP_FP8),
            "kT": kT_host.astype(NP_FP8),
            "v": v_host.astype(NP_BF16),
            "masks": mks[h].astype(NP_BF16),
        })

    bkr = run_bass_kernel_spmd(nc, in_maps, list(range(8)), trace=TRACE)
    _nc_cache["last"] = bkr
    res = bkr.results

    out = np.empty((B, T, D), np.float32)
    for core in range(8):
        b, h = core // 2, core % 2
        out[b, rows[h]] = res[core]["out_loc"].astype(np.float32)

    return out, k_full, v_full
